# revision 28
# baseline (speedup 1.0000x reference)
"""Trainium2 Bass kernel for CantorGlobalAttention (sparse attention).

Math (per direction x, expert e, batch b):
  scores[p, k] = Q[x,e,b,p] * kappa[k]          (rank-1 outer product)
  kappa[k]     = K_aff[x, route(e,w), b, q] * fac(e,w) / temp,  k=(w,q)
  attn         = softmax_k(scores)
  out[p, :]    = attn @ V_neighbors[k, :]
  final        = sum_x softmax(fusion_weights)[x] * out_x

Device strategy (8 cores, expert-parallel, 2 experts/core, 40 (x,e,b)
tuples/core):
  - scores via PE rank-1 outer products in the [k, p] (matmul-ready)
    orientation: 6 chunk matmuls per tuple -> scores [128k, 1536] fp32 in
    PSUM. Operands are bf16 hi/lo split-K decompositions
    (kappa_hi*Q_hi + kappa_hi*Q_lo + kappa_lo*Q_hi), exact to ~1e-5 while
    running at the 1 cycle/row bf16 matmul rate.
  - one ScalarE Exp per tuple, PSUM [128,1536] -> SBUF fp16, with a
    per-partition bias = -max(scores) (exact, from the rank-1 corner
    products on the host). The shift is softmax-invariant and keeps the
    exp weights in [0, 1] so fp16 storage is safe and precise.
  - attn @ V as 12 accumulating fp16 matmuls (lhsT = exp chunk [128,128],
    rhs = V chunk [128,129] with a ones column appended so the softmax
    denominator Z falls out of the same matmul, fp32 PSUM accumulation).
  - VectorE: reciprocal(Z), scale by wts[x]/Z, accumulate over x in SBUF.
  - software-pipelined emission (attn@V lags scores/exp by 2 tuples) keeps
    ScalarE -- the bottleneck engine at ~59us busy -- gap-free; V streams
    via gpsimd/SWDGE in 4-tuple batches; PE is pre-warmed against the
    p-state ramp; ACT table load is forced during startup.
  - host does all layout: neighbor gather, beta/temp folding into K,
    hi/lo splits, score maxima, fusion-weight softmax (tiny tensors only).
"""

import numpy as np
import ml_dtypes

import concourse.tile as tile
from concourse import bacc, mybir
from concourse.bass_utils import run_bass_kernel_spmd

F32 = mybir.dt.float32
BF16 = mybir.dt.bfloat16
FP16 = mybir.dt.float16
BF16_NP = ml_dtypes.bfloat16

NDIR = 5
E = 16
W = 3
D = 128
P = 256
B = 4
DEPTH = 8

N_CORES = 8
ELOC = E // N_CORES          # experts per core = 2
NT = NDIR * ELOC * B         # tuples per core = 40
NCH = W * 2                  # key chunks per tuple (w, half) = 6
KROWS = 3                   # split-K rows (hi*hi + hi*lo + lo*hi)
FREE_V = NCH * (D + 1)       # V stage free size = 774
NBLK = NT // 4               # tuple column blocks = 10


def _routes() -> np.ndarray:
    def cantor(pos: int) -> float:
        x = pos / max(1, E - 1)
        x = max(1e-06, min(x, 1.0 - 1e-06))
        val, factor = 0.0, 0.5
        for _ in range(DEPTH):
            x *= 3.0
            digit = int(x)
            x -= digit
            if digit == 2:
                val += factor
            factor *= 0.5
        return val

    coords = np.array([cantor(i) for i in range(E)], dtype=np.float32)
    routes = np.zeros((E, W), dtype=np.int32)
    for i in range(E):
        d = np.abs(coords - coords[i])
        routes[i] = np.sort(np.argsort(d, kind="stable")[:W])
    return routes


ROUTES = _routes()


def _tuple_iter():
    """(t, x, e_local, b) in x-major order (x outermost for fusion accum)."""
    t = 0
    for x in range(NDIR):
        for e in range(ELOC):
            for b in range(B):
                yield t, x, e, b
                t += 1


KQ_K0 = 0                    # k region start col in merged kq tile
KQ_Q0 = NBLK * NCH * 128     # q region start col in merged kq tile
KQ_COLS = NBLK * NCH * 128 + NBLK * 256


def _build_program():
    nc = bacc.Bacc(None)

    vd = nc.dram_tensor("v", [NBLK, 128, 4 * FREE_V], FP16, kind="ExternalInput")
    kqd = nc.dram_tensor("kq", [4, KROWS, KQ_COLS], BF16, kind="ExternalInput")
    wd = nc.dram_tensor("w", [128, NDIR], F32, kind="ExternalInput")
    md = nc.dram_tensor("m", [128, NT], F32, kind="ExternalInput")
    od = nc.dram_tensor("o", [ELOC * B, 128, 2 * 128], F32, kind="ExternalOutput")

    with tile.TileContext(nc) as tc:
        with (
            tc.tile_pool(name="const", bufs=1) as const,
            tc.tile_pool(name="vstream", bufs=4) as vpool,
            tc.tile_pool(name="exp", bufs=4) as epool,
            tc.tile_pool(name="small", bufs=4) as spool_small,
            tc.tile_pool(name="psum_s", bufs=2, space="PSUM") as pscore,
            tc.tile_pool(name="psum_o", bufs=2, space="PSUM") as pout,
        ):
            kq_tile = const.tile([128, KQ_COLS], BF16)
            wts_tile = const.tile([128, NDIR], F32)
            m_tile = const.tile([128, NT], F32)
            acc = const.tile([128, ELOC * B * 2 * 128], F32)

            # kq is tiny now (~245KB); issue first so scores can start,
            # V streams go via gpsimd (SWDGE, off the shared HWDGE path)
            nc.sync.dma_start(kq_tile[0:KROWS, :], kqd[0])
            nc.sync.dma_start(m_tile[:], md[:])
            nc.sync.dma_start(wts_tile[:], wd[:])
            for g in range(1, 4):
                nc.sync.dma_start(kq_tile[32 * g : 32 * g + KROWS, :], kqd[g])

            # warm up the PE p-state ramp while the first DMAs land: ~4us of
            # throwaway matmuls on a zeroed tile keeps the ramp model (and
            # the real HAM clock gate) at full rate when real work arrives
            warm = const.tile([32, 512], BF16)
            nc.gpsimd.memset(warm[:], 0.0)
            # dummy exp on a zeroed scrap forces the ACT table load to happen
            # during startup instead of right before the first real activation
            scrap = const.tile([32, 8], F32)
            nc.vector.memset(scrap[:], 0.0)
            nc.scalar.activation(
                scrap[:], scrap[:], mybir.ActivationFunctionType.Exp
            )
            Sw = pout.tile([128, 2, D + 1], F32, tag="O")
            for i in range(12):
                nc.tensor.matmul(
                    Sw[:, 0, :],
                    warm[0:32, 0:128],
                    warm[0:32, 0:129],
                    start=True,
                    stop=True,
                )

            def emit_tail(st):
                """main matmuls + softmax normalize + fusion accum for a tuple."""
                x, e, b, Ex, v = st
                # attended [p, d] (+ Z in col 128) accumulated over 6 chunks
                O = pout.tile([128, 2, D + 1], F32)
                for pc in range(2):
                    for c in range(NCH):
                        nc.tensor.matmul(
                            O[:, pc, :],
                            Ex[:, c * 256 + pc * 128 : c * 256 + pc * 128 + 128],
                            v[:, c * (D + 1) : (c + 1) * (D + 1)],
                            start=(c == 0),
                            stop=(c == NCH - 1),
                        )

                r = spool_small.tile([128, 2], F32)
                nc.vector.reciprocal(r[:], O[:, :, D])
                for pc in range(2):
                    idx = (e * B + b) * 2 + pc
                    dst = acc[:, idx * 128 : (idx + 1) * 128]
                    if x == 0:
                        nc.vector.tensor_scalar(
                            dst,
                            O[:, pc, 0:D],
                            r[:, pc : pc + 1],
                            wts_tile[:, x : x + 1],
                            mybir.AluOpType.mult,
                            mybir.AluOpType.mult,
                        )
                    else:
                        tmp = spool_small.tile([128, D], F32, tag="tmp")
                        nc.vector.tensor_scalar(
                            tmp[:],
                            O[:, pc, 0:D],
                            r[:, pc : pc + 1],
                            wts_tile[:, x : x + 1],
                            mybir.AluOpType.mult,
                            mybir.AluOpType.mult,
                        )
                        nc.vector.tensor_add(dst, dst, tmp[:])

                if x == NDIR - 1:
                    eb = e * B + b
                    nc.sync.dma_start(od[eb], acc[:, eb * 256 : (eb + 1) * 256])

            # Software-pipelined emission: scores/exp of tuple t+1 are emitted
            # BEFORE the attn@V matmuls of tuple t, so the PE's (shallow)
            # reorder window always has ready score work while the mains wait
            # on the exp result -- keeps ScalarE fed back-to-back.
            vt = None
            pending = []
            for t, x, e, b in _tuple_iter():
                g, blk = t // NBLK, t % NBLK
                bp = 32 * g

                if t % 4 == 0:
                    vt = vpool.tile([128, 4 * FREE_V], FP16)
                    nc.gpsimd.dma_start(vt[:], vd[t // 4])
                v = vt[:, (t % 4) * FREE_V : (t % 4 + 1) * FREE_V]

                # scores [128k, 1536]: 6 outer products, one per key chunk.
                # lhsT rows = (kappa_hi, kappa_hi, kappa_lo), rhs rows =
                # (q_hi, q_lo, q_hi): fp32-exact rank-1 product at bf16 rate.
                S = pscore.tile([128, 1536], F32)
                for c in range(NCH):
                    k0 = KQ_K0 + (blk * NCH + c) * 128
                    q0 = KQ_Q0 + blk * 256
                    nc.tensor.matmul(
                        S[:, c * 256 : (c + 1) * 256],
                        kq_tile[bp : bp + KROWS, k0 : k0 + 128],
                        kq_tile[bp : bp + KROWS, q0 : q0 + 256],
                        start=True,
                        stop=True,
                        tile_position=(bp, 0),
                    )

                # exp(s - M_t): M_t is the exact per-tuple score max (host,
                # from the rank-1 corner products). Softmax-invariant shift
                # that keeps exp weights in [~0, 1] so fp16 storage is safe.
                Ex = epool.tile([128, 1536], FP16)
                nc.scalar.activation(
                    Ex[:],
                    S[:],
                    mybir.ActivationFunctionType.Exp,
                    bias=m_tile[:, t : t + 1],
                )

                pending.append((x, e, b, Ex, v))
                while len(pending) > 2:
                    emit_tail(pending.pop(0))
            for st in pending:
                emit_tail(st)

    nc.compile()
    return nc


_PROGRAM = None


def _program():
    global _PROGRAM
    if _PROGRAM is None:
        _PROGRAM = _build_program()
    return _PROGRAM


def _hi_lo(a):
    """bf16 hi/lo split: a ~= hi + lo with hi, lo bf16."""
    hi = a.astype(BF16_NP)
    lo = (a - hi.astype(np.float32)).astype(BF16_NP)
    return hi, lo


def _prep_core_inputs(core, Q_aff, K_aff, V, beta_fac, wts_bcast):
    """Build the per-core input arrays (pure layout + tiny scalar folding)."""
    v_host = np.empty((NBLK, 128, 4 * FREE_V), dtype=np.float16)
    kq_host = np.zeros((4, KROWS, KQ_COLS), dtype=BF16_NP)
    m_host = np.zeros((128, NT), dtype=np.float32)

    for t, x, e, b in _tuple_iter():
        g, blk = t // NBLK, t % NBLK
        ge = ELOC * core + e
        q_hi, q_lo = _hi_lo(Q_aff[x, ge, b])  # [256] each
        v0 = (t % 4) * FREE_V
        qs = slice(KQ_Q0 + blk * 256, KQ_Q0 + (blk + 1) * 256)
        kq_host[g, 0, qs] = q_hi
        kq_host[g, 1, qs] = q_lo
        kq_host[g, 2, qs] = q_hi
        qrow = Q_aff[x, ge, b].astype(np.float64)
        qmin, qmax = qrow.min(), qrow.max()
        smax = -np.inf
        for c in range(NCH):
            w, half = c // 2, c % 2
            er = int(ROUTES[ge, w])
            sl = slice(half * 128, (half + 1) * 128)
            v_host[t // 4, :, v0 + c * (D + 1) : v0 + c * (D + 1) + D] = V[
                x, er, b, sl, :
            ]
            v_host[t // 4, :, v0 + c * (D + 1) + D] = 1.0
            kappa = K_aff[x, er, b, sl] * beta_fac[ge, w]
            k_hi, k_lo = _hi_lo(kappa)
            ks = slice(KQ_K0 + (blk * NCH + c) * 128, KQ_K0 + (blk * NCH + c + 1) * 128)
            kq_host[g, 0, ks] = k_hi
            kq_host[g, 1, ks] = k_hi
            kq_host[g, 2, ks] = k_lo
            kmin, kmax = float(kappa.min()), float(kappa.max())
            smax = max(
                smax, kmax * qmax, kmax * qmin, kmin * qmax, kmin * qmin
            )
        m_host[:, t] = -np.float32(smax)
    return {"v": v_host, "kq": kq_host, "w": wts_bcast, "m": m_host}


def kernel(Q_aff, K_aff, V, betas, temperature, fusion_weights):
    Q_aff = np.asarray(Q_aff, dtype=np.float32)
    K_aff = np.asarray(K_aff, dtype=np.float32)
    V = np.asarray(V, dtype=np.float32)
    betas = np.asarray(betas, dtype=np.float32)
    temperature = np.asarray(temperature, dtype=np.float32)
    fusion_weights = np.asarray(fusion_weights, dtype=np.float32)

    temp = abs(float(temperature[0])) + 1e-06
    # fac(e, w) = sigmoid(betas[e, route]) for cross edges, 1 for self; /temp
    sig = 1.0 / (1.0 + np.exp(-betas.astype(np.float64)))
    beta_fac = np.empty((E, W), dtype=np.float64)
    for e in range(E):
        for w in range(W):
            er = int(ROUTES[e, w])
            beta_fac[e, w] = (1.0 if er == e else sig[e, er]) / temp
    beta_fac = beta_fac.astype(np.float32)

    fw = fusion_weights.astype(np.float64)
    fw = np.exp(fw - fw.max())
    wts = (fw / fw.sum()).astype(np.float32)
    wts_bcast = np.broadcast_to(wts, (128, NDIR)).copy()

    nc = _program()
    in_maps = [
        _prep_core_inputs(c, Q_aff, K_aff, V, beta_fac, wts_bcast)
        for c in range(N_CORES)
    ]
    res = run_bass_kernel_spmd(nc, in_maps, list(range(N_CORES)))

    out = np.empty((B, E * P, D), dtype=np.float32)
    for c in range(N_CORES):
        o = res.results[c]["o"]  # [ELOC*B, 128(p), 2*128]
        for e in range(ELOC):
            ge = ELOC * c + e
            # o[e*B+b][p, pc*128 + d] -> out[b, ge*P + pc*128 + p, d]
            oe = o[e * B : (e + 1) * B].reshape(B, 128, 2, 128)
            out[:, ge * P : (ge + 1) * P, :] = oe.transpose(0, 2, 1, 3).reshape(
                B, P, D
            )
    return out
